# revision 3
# baseline (speedup 1.0000x reference)
"""2-layer GRU (B=64, T=256, D=64, H=1024) + final linear on TRN2, 8 cores.

Strategy: data-parallel over batch (8 rows per core, no collectives).
Per core, each GRU step runs the [8,1024]x[1024,3072] gate matmuls as four
concurrent col-tiled PE streams (tile_position (0,32g)), with the hidden
state kept both in a stacked [128,256] fp32 layout (partition block g =
h columns [256g,256g+256)) for the elementwise cell update, and as bf16
transposed chunks (via PE transpose) that feed the next step's matmuls as
stationary operands.  x-projections and biases are folded into the same
PSUM accumulation as extra K-chunks.  The T=256 loop is a Tile For_i with
a 4-step unrolled body.
"""
import numpy as np
import ml_dtypes

import concourse.bass as bass
import concourse.tile as tile
from concourse import bacc, mybir

F32 = mybir.dt.float32
BF16 = mybir.dt.bfloat16
AF = mybir.ActivationFunctionType
ALU = mybir.AluOpType

B = 8           # batch rows per core
H = 1024
KC = H // 128   # 8 K-chunks
Q = H // 4      # 256
G3 = 3 * H
T = 256
N_CORES = 8

_cache = {}


def _build(unroll=4, repeat=1):
    nc = bacc.Bacc("TRN2", target_bir_lowering=False, debug=False,
                   enable_asserts=False, num_devices=N_CORES)

    xT_d = nc.dram_tensor("xT", [T, 128, B], BF16, kind="ExternalInput")
    Wh0_d = nc.dram_tensor("Wh0", [KC, 128, G3], BF16, kind="ExternalInput")
    Wx0_d = nc.dram_tensor("Wx0", [128, G3], BF16, kind="ExternalInput")
    bhn0_d = nc.dram_tensor("bhn0", [128, H], BF16, kind="ExternalInput")
    Wh1_d = nc.dram_tensor("Wh1", [KC, 128, G3], BF16, kind="ExternalInput")
    Wi1_d = nc.dram_tensor("Wi1", [KC, 128, G3], BF16, kind="ExternalInput")
    ba1_d = nc.dram_tensor("ba1", [128, G3], BF16, kind="ExternalInput")
    bb1_d = nc.dram_tensor("bb1", [128, H], BF16, kind="ExternalInput")
    fcw_d = nc.dram_tensor("fcw", [KC, 128, 1], BF16, kind="ExternalInput")
    id_d = nc.dram_tensor("ident", [128, 128], F32, kind="ExternalInput")
    out_d = nc.dram_tensor("out", [B, 1], F32, kind="ExternalOutput")

    with tile.TileContext(nc) as tc:
        with (
            tc.tile_pool(name="weights", bufs=1) as wpool,
            tc.tile_pool(name="state", bufs=1) as spool,
            tc.tile_pool(name="work", bufs=3) as work,
            tc.tile_pool(name="pgates", bufs=1, space="PSUM") as pg,
            tc.tile_pool(name="ptrans", bufs=2, space="PSUM") as ptp,
        ):
            Wh0 = wpool.tile([128, KC, G3], BF16, tag="Wh0")
            nc.sync.dma_start(Wh0[:], Wh0_d.ap().rearrange("k p n -> p k n"))
            Wx0 = wpool.tile([128, G3], BF16, tag="Wx0")
            nc.sync.dma_start(Wx0[:], Wx0_d.ap())
            bhn0 = wpool.tile([128, H], BF16, tag="bhn0")
            nc.sync.dma_start(bhn0[:], bhn0_d.ap())
            Wh1 = wpool.tile([128, KC, G3], BF16, tag="Wh1")
            nc.sync.dma_start(Wh1[:], Wh1_d.ap().rearrange("k p n -> p k n"))
            Wi1 = wpool.tile([128, KC, G3], BF16, tag="Wi1")
            nc.sync.dma_start(Wi1[:], Wi1_d.ap().rearrange("k p n -> p k n"))
            ba1 = wpool.tile([128, G3], BF16, tag="ba1")
            nc.sync.dma_start(ba1[:], ba1_d.ap())
            bb1 = wpool.tile([128, H], BF16, tag="bb1")
            nc.sync.dma_start(bb1[:], bb1_d.ap())
            fcw = wpool.tile([128, KC, 1], BF16, tag="fcw")
            nc.sync.dma_start(fcw[:], fcw_d.ap().rearrange("k p n -> p k n"))
            idf = wpool.tile([128, 128], F32, tag="idf")
            nc.sync.dma_start(idf[:], id_d.ap())
            xT = wpool.tile([128, T, B], BF16, tag="xT")
            nc.sync.dma_start(xT[:], xT_d.ap().rearrange("t p b -> p t b"))
            ones = wpool.tile([128, B], BF16, tag="ones")
            nc.gpsimd.memset(ones[:], 0.0)
            nc.gpsimd.memset(ones[0:1, :], 1.0)

            h0s = spool.tile([128, Q], F32, tag="h0s")
            h2s = spool.tile([128, Q], F32, tag="h2s")
            hT0 = spool.tile([128, 2, 128], BF16, tag="hT0")
            hT2 = spool.tile([128, 2, 128], BF16, tag="hT2")
            nc.gpsimd.memset(h0s[:], 0.0)
            nc.gpsimd.memset(h2s[:], 0.0)
            nc.gpsimd.memset(hT0[:], 0.0)
            nc.gpsimd.memset(hT2[:], 0.0)

            def hT_chunk(hT, k):
                return hT[:, k % 2, 32 * (k // 2):32 * (k // 2) + B]

            def mm_group(p_ap_fn, hTs, W, col0, start, stop, ks=range(KC)):
                """K-chunk matmuls of W cols [col0+Qg, col0+Qg+Q) into psum."""
                for k in ks:
                    for g in range(4):
                        nc.tensor.matmul(
                            p_ap_fn(g), hT_chunk(hTs, k),
                            W[:, k, col0 + Q * g: col0 + Q * g + Q],
                            start=(start and k == ks[0]), stop=False,
                            tile_position=(0, 32 * g))
                _ = stop  # stop carried by a later bias/x matmul

            def bias_mms(p_ap_fn, lhsT, W2d, col0, start, stop):
                for g in range(4):
                    nc.tensor.matmul(
                        p_ap_fn(g), lhsT[:],
                        W2d[:, col0 + Q * g: col0 + Q * g + Q],
                        start=start, stop=stop, tile_position=(0, 32 * g))

            def chain_tail(hss, hTs, nt, d, z_psum):
                """sigma(z) -> t2 -> h' -> transpose -> evac."""
                z_t = work.tile([128, Q], F32, tag="z_t")
                nc.scalar.activation(z_t[:], z_psum[:], AF.Sigmoid)
                t2 = work.tile([128, Q], F32, tag="t2")
                nc.vector.scalar_tensor_tensor(t2[:], d[:], 1.0, z_t[:],
                                               op0=ALU.mult, op1=ALU.mult)
                nc.vector.scalar_tensor_tensor(hss[:], t2[:], 0.0, nt[:],
                                               op0=ALU.add, op1=ALU.add)
                pt = ptp.tile([128, 2 * 128], F32, tag="pt")
                for half in range(2):
                    nc.tensor.transpose(pt[:, 128 * half:128 * half + 128],
                                        hss[:, 128 * half:128 * half + 128], idf[:])
                nc.scalar.activation(hTs[:, :, :], pt[:], AF.Copy)

            def emit_step(xst):
                pn0 = pg.tile([128, 2 * Q], F32, tag="pn0")
                pr0 = pg.tile([128, Q], F32, tag="pr0")
                pz0 = pg.tile([128, Q], F32, tag="pz0")
                pn1 = pg.tile([128, 2 * Q], F32, tag="pn1")
                pr1 = pg.tile([128, Q], F32, tag="pr1")
                pz1 = pg.tile([128, Q], F32, tag="pz1")

                # ---------------- layer 0 ----------------
                # n1_0 = h0 @ Whh_n + b_hh_n
                mm_group(lambda g: pn0[32 * g:32 * g + B, 0:Q], hT0, Wh0,
                         2 * H, True, False)
                bias_mms(lambda g: pn0[32 * g:32 * g + B, 0:Q], ones, bhn0,
                         0, False, True)
                # r_0 = h0 @ Whh_r + x-part
                mm_group(lambda g: pr0[32 * g:32 * g + B, :], hT0, Wh0,
                         0, True, False)
                bias_mms(lambda g: pr0[32 * g:32 * g + B, :], xst, Wx0,
                         0, False, True)
                r_t = work.tile([128, Q], F32, tag="r_t")
                nc.scalar.activation(r_t[:], pr0[:], AF.Sigmoid)
                t1 = work.tile([128, Q], F32, tag="t1")
                nc.vector.scalar_tensor_tensor(t1[:], pn0[:, 0:Q], 1.0, r_t[:],
                                               op0=ALU.mult, op1=ALU.mult)
                # --- L1 filler A: n1_1 h2-part (+ bias) ---
                mm_group(lambda g: pn1[32 * g:32 * g + B, 0:Q], hT2, Wh1,
                         2 * H, True, False)
                bias_mms(lambda g: pn1[32 * g:32 * g + B, 0:Q], ones,
                         ba1, 2 * H, False, True)
                # --- L0 n2 (x-part of n) ---
                bias_mms(lambda g: pn0[32 * g:32 * g + B, Q:2 * Q], xst, Wx0,
                         2 * H, True, True)
                pre_n = work.tile([128, Q], F32, tag="pre_n")
                nc.vector.scalar_tensor_tensor(pre_n[:], t1[:], 0.0,
                                               pn0[:, Q:2 * Q],
                                               op0=ALU.add, op1=ALU.add)
                n_t0 = work.tile([128, Q], F32, tag="n_t0")
                nc.scalar.activation(n_t0[:], pre_n[:], AF.Tanh)
                d0 = work.tile([128, Q], F32, tag="d0")
                nc.vector.scalar_tensor_tensor(d0[:], n_t0[:], -1.0, h0s[:],
                                               op0=ALU.mult, op1=ALU.add)
                # --- L0 z ---
                mm_group(lambda g: pz0[32 * g:32 * g + B, :], hT0, Wh0,
                         H, True, False)
                bias_mms(lambda g: pz0[32 * g:32 * g + B, :], xst, Wx0,
                         H, False, True)
                chain_tail(h0s, hT0, n_t0, d0, pz0)   # -> hT0 = h1_t

                # --- L1 filler B: r,z h2-parts ---
                mm_group(lambda g: pr1[32 * g:32 * g + B, :], hT2, Wh1,
                         0, True, False)
                mm_group(lambda g: pz1[32 * g:32 * g + B, :], hT2, Wh1,
                         H, True, False)

                # ---------------- layer 1 (h1 parts) ----------------
                mm_group(lambda g: pr1[32 * g:32 * g + B, :], hT0, Wi1,
                         0, False, False)
                bias_mms(lambda g: pr1[32 * g:32 * g + B, :], ones, ba1,
                         0, False, True)
                r_t1 = work.tile([128, Q], F32, tag="r_t1")
                nc.scalar.activation(r_t1[:], pr1[:], AF.Sigmoid)
                t11 = work.tile([128, Q], F32, tag="t11")
                nc.vector.scalar_tensor_tensor(t11[:], pn1[:, 0:Q], 1.0, r_t1[:],
                                               op0=ALU.mult, op1=ALU.mult)
                mm_group(lambda g: pn1[32 * g:32 * g + B, Q:2 * Q], hT0, Wi1,
                         2 * H, True, False)
                bias_mms(lambda g: pn1[32 * g:32 * g + B, Q:2 * Q], ones, bb1,
                         0, False, True)
                pre_n1 = work.tile([128, Q], F32, tag="pre_n1")
                nc.vector.scalar_tensor_tensor(pre_n1[:], t11[:], 0.0,
                                               pn1[:, Q:2 * Q],
                                               op0=ALU.add, op1=ALU.add)
                n_t1 = work.tile([128, Q], F32, tag="n_t1")
                nc.scalar.activation(n_t1[:], pre_n1[:], AF.Tanh)
                d1 = work.tile([128, Q], F32, tag="d1")
                nc.vector.scalar_tensor_tensor(d1[:], n_t1[:], -1.0, h2s[:],
                                               op0=ALU.mult, op1=ALU.add)
                mm_group(lambda g: pz1[32 * g:32 * g + B, :], hT0, Wi1,
                         H, False, False)
                bias_mms(lambda g: pz1[32 * g:32 * g + B, :], ones, ba1,
                         H, False, True)
                chain_tail(h2s, hT2, n_t1, d1, pz1)

            U = unroll

            def t_loop():
                with tc.For_i(0, T // U, 1) as iv:
                    for u in range(U):
                        xst = work.tile([128, B], BF16, tag=f"xst{u}")
                        nc.scalar.activation(
                            xst[:], xT[:, bass.ds(iv * U + u, 1), :].opt(),
                            AF.Copy)
                        emit_step(xst)

            if repeat == 1:
                t_loop()
            else:
                with tc.For_i(0, repeat, 1):
                    t_loop()

            p_fc = ptp.tile([B, 1], F32, tag="pt")
            for k in range(KC):
                nc.tensor.matmul(p_fc[:], hT_chunk(hT2, k), fcw[:, k],
                                 start=(k == 0), stop=(k == KC - 1))
            ov = work.tile([B, 1], F32, tag="ov")
            nc.vector.tensor_copy(ov[:], p_fc[:])
            nc.sync.dma_start(out_d.ap(), ov[:])

    nc.compile()
    return nc


def _prep_inputs(x, w_ih_l0, w_hh_l0, b_ih_l0, b_hh_l0,
                 w_ih_l1, w_hh_l1, b_ih_l1, b_hh_l1, fc_w, fc_b):
    bf = ml_dtypes.bfloat16
    f32 = np.float32
    x = np.asarray(x, f32)
    w_ih_l0 = np.asarray(w_ih_l0, f32); w_hh_l0 = np.asarray(w_hh_l0, f32)
    b_ih_l0 = np.asarray(b_ih_l0, f32); b_hh_l0 = np.asarray(b_hh_l0, f32)
    w_ih_l1 = np.asarray(w_ih_l1, f32); w_hh_l1 = np.asarray(w_hh_l1, f32)
    b_ih_l1 = np.asarray(b_ih_l1, f32); b_hh_l1 = np.asarray(b_hh_l1, f32)
    fc_w = np.asarray(fc_w, f32)

    Wh0 = w_hh_l0.T.reshape(KC, 128, G3).astype(bf)
    Wh1 = w_hh_l1.T.reshape(KC, 128, G3).astype(bf)
    Wi1 = w_ih_l1.T.reshape(KC, 128, G3).astype(bf)

    Wx0 = np.zeros((128, G3), f32)
    Wx0[0:64] = w_ih_l0.T
    Wx0[64] = np.concatenate([(b_ih_l0 + b_hh_l0)[0:2 * H], b_ih_l0[2 * H:3 * H]])
    Wx0 = Wx0.astype(bf)
    bhn0 = np.zeros((128, H), f32); bhn0[0] = b_hh_l0[2 * H:3 * H]
    bhn0 = bhn0.astype(bf)
    ba1 = np.zeros((128, G3), f32)
    ba1[0] = np.concatenate([(b_ih_l1 + b_hh_l1)[0:2 * H], b_hh_l1[2 * H:3 * H]])
    ba1 = ba1.astype(bf)
    bb1 = np.zeros((128, H), f32); bb1[0] = b_ih_l1[2 * H:3 * H]
    bb1 = bb1.astype(bf)
    fcw = fc_w.T.reshape(KC, 128, 1).astype(bf)
    ident = np.eye(128, dtype=f32)

    shared = dict(Wh0=Wh0, Wx0=Wx0, bhn0=bhn0, Wh1=Wh1, Wi1=Wi1,
                  ba1=ba1, bb1=bb1, fcw=fcw, ident=ident)
    in_maps = []
    for c in range(N_CORES):
        xs = x[c * B:(c + 1) * B]                 # [B, T, D]
        xTc = np.zeros((T, 128, B), f32)
        xTc[:, 0:64, :] = xs.transpose(1, 2, 0)
        xTc[:, 64, :] = 1.0
        m = dict(shared)
        m["xT"] = xTc.astype(bf)
        in_maps.append(m)
    return in_maps


def kernel(**inputs) -> np.ndarray:
    from concourse import bass_utils
    if "nc" not in _cache:
        _cache["nc"] = _build()
    nc = _cache["nc"]
    in_maps = _prep_inputs(**inputs)
    res = bass_utils.run_bass_kernel_spmd(nc, in_maps,
                                          core_ids=list(range(N_CORES)))
    out = np.concatenate([res.results[c]["out"] for c in range(N_CORES)], axis=0)
    return (out + np.asarray(inputs["fc_b"], np.float32)).astype(np.float32)



# revision 13
# speedup vs baseline: 1.2840x; 1.2840x over previous
"""2-layer GRU (B=64, T=256, D=64, H=1024) + final linear on TRN2, 8 cores.

Strategy: data-parallel over batch (8 rows per core, no collectives).
Per core, each GRU step runs the [8,1024]x[1024,3072] gate matmuls as four
concurrent col-tiled PE streams (tile_position (0,32g)).  Gate columns are
laid out band-local as [r|z (512) | n (256)] per band so r+z stream as one
N=512 matmul per k-chunk per band.  The hidden state is kept both stacked
[128,256] fp32 (partition 32g+b = batch b, band g = h cols [256g,256g+256))
for the elementwise cell update, and as bf16 transposed chunks (via PE
transpose) that feed the next step's matmuls as stationary operands.
x-projections and biases are folded into the same PSUM accumulation as
extra moving passes.  The T loop is a Tile For_i with an unrolled body.
"""
import numpy as np
import ml_dtypes

import concourse.bass as bass
import concourse.tile as tile
from concourse import bacc, mybir

F32 = mybir.dt.float32
BF16 = mybir.dt.bfloat16
AF = mybir.ActivationFunctionType
ALU = mybir.AluOpType

B = 8           # batch rows per core
H = 1024
KC = H // 128   # 8 K-chunks
Q = H // 4      # 256 h-cols per band
G3 = 3 * H
T = 256
N_CORES = 8

_cache = {}


def _build(unroll=4, repeat=1, n_t=T):
    nc = bacc.Bacc("TRN2", target_bir_lowering=False, debug=False,
                   enable_asserts=False, num_devices=N_CORES)

    xT_d = nc.dram_tensor("xT", [T, 128, B], BF16, kind="ExternalInput")
    # Per-layer recurrent weights, band-local: [k, :, g, 0:512]=r|z,
    # [k, :, g, 512:768]=n for h-cols [256g, 256g+256).
    W0_d = nc.dram_tensor("W0", [KC, 128, 4 * 768], BF16, kind="ExternalInput")
    Wh1_d = nc.dram_tensor("Wh1", [KC, 128, 4 * 768], BF16, kind="ExternalInput")
    Wi1_d = nc.dram_tensor("Wi1", [KC, 128, 4 * 768], BF16, kind="ExternalInput")
    # x/bias moving operands (stationary = xst or ones).
    Wx0_d = nc.dram_tensor("Wx0", [128, 4 * 768], BF16, kind="ExternalInput")
    bhn0_d = nc.dram_tensor("bhn0", [128, 4 * 256], BF16, kind="ExternalInput")
    ba1_d = nc.dram_tensor("ba1", [128, 4 * 768], BF16, kind="ExternalInput")
    bb1_d = nc.dram_tensor("bb1", [128, 4 * 256], BF16, kind="ExternalInput")
    fcw_d = nc.dram_tensor("fcw", [KC, 128, 1], BF16, kind="ExternalInput")
    id_d = nc.dram_tensor("ident", [128, 128], F32, kind="ExternalInput")
    out_d = nc.dram_tensor("out", [B, 1], F32, kind="ExternalOutput")

    with tile.TileContext(nc) as tc:
        with (
            tc.tile_pool(name="weights", bufs=1) as wpool,
            tc.tile_pool(name="state", bufs=1) as spool,
            tc.tile_pool(name="work", bufs=3) as work,
            tc.tile_pool(name="pgates", bufs=1, space="PSUM") as pg,
            tc.tile_pool(name="ptrans", bufs=2, space="PSUM") as ptp,
        ):
            W0 = wpool.tile([128, KC, 4 * 768], BF16, tag="W0")
            nc.sync.dma_start(W0[:], W0_d.ap().rearrange("k p n -> p k n"))
            Wh1 = wpool.tile([128, KC, 4 * 768], BF16, tag="Wh1")
            nc.sync.dma_start(Wh1[:], Wh1_d.ap().rearrange("k p n -> p k n"))
            Wi1 = wpool.tile([128, KC, 4 * 768], BF16, tag="Wi1")
            nc.sync.dma_start(Wi1[:], Wi1_d.ap().rearrange("k p n -> p k n"))
            Wx0 = wpool.tile([128, 4 * 768], BF16, tag="Wx0")
            nc.sync.dma_start(Wx0[:], Wx0_d.ap())
            bhn0 = wpool.tile([128, 4 * 256], BF16, tag="bhn0")
            nc.sync.dma_start(bhn0[:], bhn0_d.ap())
            ba1 = wpool.tile([128, 4 * 768], BF16, tag="ba1")
            nc.sync.dma_start(ba1[:], ba1_d.ap())
            bb1 = wpool.tile([128, 4 * 256], BF16, tag="bb1")
            nc.sync.dma_start(bb1[:], bb1_d.ap())
            fcw = wpool.tile([128, KC, 1], BF16, tag="fcw")
            nc.sync.dma_start(fcw[:], fcw_d.ap().rearrange("k p n -> p k n"))
            idf = wpool.tile([128, 128], F32, tag="idf")
            nc.sync.dma_start(idf[:], id_d.ap())
            xT = wpool.tile([128, T, B], BF16, tag="xT")
            nc.sync.dma_start(xT[:], xT_d.ap().rearrange("t p b -> p t b"))
            ones = wpool.tile([128, B], BF16, tag="ones")
            nc.gpsimd.memset(ones[:], 0.0)
            nc.gpsimd.memset(ones[0:1, :], 1.0)

            h0s = spool.tile([128, Q], F32, tag="h0s")
            h2s = spool.tile([128, Q], F32, tag="h2s")
            hT0 = spool.tile([128, 2, 128], BF16, tag="hT0")
            hT2 = spool.tile([128, 2, 128], BF16, tag="hT2")
            nc.gpsimd.memset(h0s[:], 0.0)
            nc.gpsimd.memset(h2s[:], 0.0)
            nc.gpsimd.memset(hT0[:], 0.0)
            nc.gpsimd.memset(hT2[:], 0.0)

            def hT_chunk(hT, k):
                return hT[:, k % 2, 32 * (k // 2):32 * (k // 2) + B]

            def ones_pass(pnh, W2d):
                for g in range(4):
                    nc.tensor.matmul(
                        pnh[32 * g:32 * g + B, :], ones[:],
                        W2d[:, 256 * g:256 * g + 256],
                        start=False, stop=True, tile_position=(0, 32 * g))

            def chain_ew(hss, prz, pnh, pnx, rzt):
                """gates -> h' (elementwise only; ACT+DVE)."""
                nc.scalar.activation(rzt[:], prz[:], AF.Sigmoid)
                t1 = work.tile([128, Q], F32, tag="t1")
                nc.vector.scalar_tensor_tensor(t1[:], pnh[:], 1.0, rzt[:, 0:Q],
                                               op0=ALU.mult, op1=ALU.mult)
                pre_n = work.tile([128, Q], F32, tag="pre_n")
                nc.vector.scalar_tensor_tensor(pre_n[:], t1[:], 0.0, pnx[:],
                                               op0=ALU.add, op1=ALU.add)
                n_t = work.tile([128, Q], F32, tag="n_t")
                nc.scalar.activation(n_t[:], pre_n[:], AF.Tanh)
                d = work.tile([128, Q], F32, tag="d")
                nc.vector.scalar_tensor_tensor(d[:], n_t[:], -1.0, hss[:],
                                               op0=ALU.mult, op1=ALU.add)
                t2 = work.tile([128, Q], F32, tag="t2")
                nc.vector.scalar_tensor_tensor(t2[:], d[:], 1.0,
                                               rzt[:, Q:2 * Q],
                                               op0=ALU.mult, op1=ALU.mult)
                nc.vector.scalar_tensor_tensor(hss[:], t2[:], 0.0, n_t[:],
                                               op0=ALU.add, op1=ALU.add)

            def trans_copy(hss, hTs):
                """h' -> PE transpose -> hT (bf16), even chunks first."""
                pt = ptp.tile([128, 2 * 128], F32, tag="pt")
                for half in range(2):
                    nc.tensor.transpose(pt[:, 128 * half:128 * half + 128],
                                        hss[:, 128 * half:128 * half + 128],
                                        idf[:])
                nc.scalar.activation(hTs[:, 0, :], pt[:, 0:128], AF.Copy)
                nc.scalar.activation(hTs[:, 1, :], pt[:, 128:256], AF.Copy)

            def emit_step(xst):
                prz0 = pg.tile([128, 512], F32, tag="prz0")
                pnh0 = pg.tile([128, Q], F32, tag="pnh0")
                pnx0 = pg.tile([128, Q], F32, tag="pnx0")
                prz1 = pg.tile([128, 512], F32, tag="prz1")
                pnh1 = pg.tile([128, Q], F32, tag="pnh1")
                pnx1 = pg.tile([128, Q], F32, tag="pnx1")

                # A: layer-0 streams.  rz group closes early (x-rz right
                # after the rz k-loop) so the sigmoid can start while the
                # nh/nx streams are still on the PE.
                for k in range(KC):
                    for g in range(4):
                        nc.tensor.matmul(
                            prz0[32 * g:32 * g + B, :], hT_chunk(hT0, k),
                            W0[:, k, 768 * g:768 * g + 512],
                            start=(k == 0), stop=False,
                            tile_position=(0, 32 * g))
                for g in range(4):
                    nc.tensor.matmul(
                        prz0[32 * g:32 * g + B, :], xst[:],
                        Wx0[:, 768 * g:768 * g + 512],
                        start=False, stop=True, tile_position=(0, 32 * g))
                for k in range(KC):
                    for g in range(4):
                        nc.tensor.matmul(
                            pnh0[32 * g:32 * g + B, :], hT_chunk(hT0, k),
                            W0[:, k, 768 * g + 512:768 * g + 768],
                            start=(k == 0), stop=False,
                            tile_position=(0, 32 * g))
                for g in range(4):
                    nc.tensor.matmul(
                        pnx0[32 * g:32 * g + B, :], xst[:],
                        Wx0[:, 768 * g + 512:768 * g + 768],
                        start=True, stop=True, tile_position=(0, 32 * g))
                ones_pass(pnh0, bhn0)

                # B: transpose h2_{t-1} into hT2 (h2s still holds step t-1's
                # state here; its step-t update is emitted later).
                trans_copy(h2s, hT2)

                # L0 elementwise (ACT/DVE; overlaps C on the PE)
                rz0 = work.tile([128, 512], F32, tag="rz0")
                chain_ew(h0s, prz0, pnh0, pnx0, rz0)

                # C1: layer-1 recurrent rz stream (PE filler under L0 chain)
                for k in range(KC):
                    for g in range(4):
                        nc.tensor.matmul(
                            prz1[32 * g:32 * g + B, :], hT_chunk(hT2, k),
                            Wh1[:, k, 768 * g:768 * g + 512],
                            start=(k == 0), stop=False,
                            tile_position=(0, 32 * g))

                # D: transpose h1_t into hT0 (needs L0 chain done)
                trans_copy(h0s, hT0)

                # C2: layer-1 recurrent nh stream + bias
                for k in range(KC):
                    for g in range(4):
                        nc.tensor.matmul(
                            pnh1[32 * g:32 * g + B, :], hT_chunk(hT2, k),
                            Wh1[:, k, 768 * g + 512:768 * g + 768],
                            start=(k == 0), stop=False,
                            tile_position=(0, 32 * g))
                ones_pass(pnh1, bb1)

                # E: layer-1 input rz stream (h1_t @ Wi1), even chunks first
                for k in (0, 2, 4, 6, 1, 3, 5, 7):
                    for g in range(4):
                        nc.tensor.matmul(
                            prz1[32 * g:32 * g + B, :], hT_chunk(hT0, k),
                            Wi1[:, k, 768 * g:768 * g + 512],
                            start=False, stop=False, tile_position=(0, 32 * g))
                for g in range(4):
                    nc.tensor.matmul(
                        prz1[32 * g:32 * g + B, :], ones[:],
                        ba1[:, 768 * g:768 * g + 512],
                        start=False, stop=True, tile_position=(0, 32 * g))

                # F: layer-1 input n stream
                for k in (0, 2, 4, 6, 1, 3, 5, 7):
                    for g in range(4):
                        nc.tensor.matmul(
                            pnx1[32 * g:32 * g + B, :], hT_chunk(hT0, k),
                            Wi1[:, k, 768 * g + 512:768 * g + 768],
                            start=(k == 0), stop=False, tile_position=(0, 32 * g))
                for g in range(4):
                    nc.tensor.matmul(
                        pnx1[32 * g:32 * g + B, :], ones[:],
                        ba1[:, 768 * g + 512:768 * g + 768],
                        start=False, stop=True, tile_position=(0, 32 * g))

                # L1 elementwise -> h2s (transposed at the start of step t+1)
                rz1 = work.tile([128, 512], F32, tag="rz1")
                chain_ew(h2s, prz1, pnh1, pnx1, rz1)

            U = unroll

            def t_loop():
                with tc.For_i(0, n_t // U, 1) as iv:
                    for u in range(U):
                        xst = work.tile([128, B], BF16, tag=f"xst{u}")
                        nc.scalar.activation(
                            xst[:], xT[:, bass.ds(iv * U + u, 1), :].opt(),
                            AF.Copy)
                        emit_step(xst)

            if repeat == 1:
                t_loop()
            else:
                with tc.For_i(0, repeat, 1):
                    t_loop()

            trans_copy(h2s, hT2)
            p_fc = ptp.tile([B, 1], F32, tag="pt")
            for k in range(KC):
                nc.tensor.matmul(p_fc[:], hT_chunk(hT2, k), fcw[:, k],
                                 start=(k == 0), stop=(k == KC - 1))
            ov = work.tile([B, 1], F32, tag="ov")
            nc.vector.tensor_copy(ov[:], p_fc[:])
            nc.sync.dma_start(out_d.ap(), ov[:])

    nc.compile()
    return nc


def _band_local_768(Wt):
    """[1024(k-dims flattened later), 3072] -> cols band-local [4, 768].

    Wt: [H, 3H] (= W.T with gate order r|z|n).  Returns [H, 4, 768] where
    [:, g, 0:256]=r cols 256g:256g+256, [:, g, 256:512]=z, [:, g, 512:768]=n.
    """
    Hd = Wt.shape[0]
    out = np.empty((Hd, 4, 768), Wt.dtype)
    for g in range(4):
        out[:, g, 0:256] = Wt[:, 256 * g:256 * g + 256]
        out[:, g, 256:512] = Wt[:, H + 256 * g:H + 256 * g + 256]
        out[:, g, 512:768] = Wt[:, 2 * H + 256 * g:2 * H + 256 * g + 256]
    return out


def _band_local_256(v):
    """[3H] bias -> n-part band-local [4, 256] is just a reshape of v[2H:]."""
    return v[2 * H:3 * H].reshape(4, 256)


def _prep_inputs(x, w_ih_l0, w_hh_l0, b_ih_l0, b_hh_l0,
                 w_ih_l1, w_hh_l1, b_ih_l1, b_hh_l1, fc_w, fc_b):
    bf = ml_dtypes.bfloat16
    f32 = np.float32
    x = np.asarray(x, f32)
    w_ih_l0 = np.asarray(w_ih_l0, f32); w_hh_l0 = np.asarray(w_hh_l0, f32)
    b_ih_l0 = np.asarray(b_ih_l0, f32); b_hh_l0 = np.asarray(b_hh_l0, f32)
    w_ih_l1 = np.asarray(w_ih_l1, f32); w_hh_l1 = np.asarray(w_hh_l1, f32)
    b_ih_l1 = np.asarray(b_ih_l1, f32); b_hh_l1 = np.asarray(b_hh_l1, f32)
    fc_w = np.asarray(fc_w, f32)

    def rec_w(Whh):  # [3H, H] -> [KC, 128, 4*768] band-local
        bl = _band_local_768(Whh.T)                  # [H, 4, 768]
        return bl.reshape(KC, 128, 4 * 768).astype(bf)

    W0 = rec_w(w_hh_l0)
    Wh1 = rec_w(w_hh_l1)
    Wi1 = rec_w(w_ih_l1)

    # Wx0: stationary xst = [x_t (rows 0:64) | 1.0 (row 64)].
    # rz cols: x-part + (b_ih+b_hh)_rz at row 64; n cols: x-part + b_ih_n.
    Wx0_full = np.zeros((128, G3), f32)
    Wx0_full[0:64] = w_ih_l0.T
    Wx0_full[64] = np.concatenate([(b_ih_l0 + b_hh_l0)[0:2 * H],
                                   b_ih_l0[2 * H:3 * H]])
    Wx0 = _band_local_768(Wx0_full)                  # [128, 4, 768]
    Wx0 = Wx0.reshape(128, 4 * 768).astype(bf)

    bhn0 = np.zeros((128, 4, 256), f32)
    bhn0[0] = _band_local_256(b_hh_l0)
    bhn0 = bhn0.reshape(128, 4 * 256).astype(bf)

    ba1_full = np.zeros((128, G3), f32)
    ba1_full[0] = np.concatenate([(b_ih_l1 + b_hh_l1)[0:2 * H],
                                  b_ih_l1[2 * H:3 * H]])
    ba1 = _band_local_768(ba1_full).reshape(128, 4 * 768).astype(bf)

    bb1 = np.zeros((128, 4, 256), f32)
    bb1[0] = _band_local_256(b_hh_l1)
    bb1 = bb1.reshape(128, 4 * 256).astype(bf)

    fcw = fc_w.T.reshape(KC, 128, 1).astype(bf)
    ident = np.eye(128, dtype=f32)

    shared = dict(W0=W0, Wx0=Wx0, bhn0=bhn0, Wh1=Wh1, Wi1=Wi1,
                  ba1=ba1, bb1=bb1, fcw=fcw, ident=ident)
    in_maps = []
    for c in range(N_CORES):
        xs = x[c * B:(c + 1) * B]                 # [B, T, D]
        xTc = np.zeros((T, 128, B), f32)
        xTc[:, 0:64, :] = xs.transpose(1, 2, 0)
        xTc[:, 64, :] = 1.0
        m = dict(shared)
        m["xT"] = xTc.astype(bf)
        in_maps.append(m)
    return in_maps


def kernel(**inputs) -> np.ndarray:
    from concourse import bass_utils
    if "nc" not in _cache:
        _cache["nc"] = _build()
    nc = _cache["nc"]
    in_maps = _prep_inputs(**inputs)
    res = bass_utils.run_bass_kernel_spmd(nc, in_maps,
                                          core_ids=list(range(N_CORES)))
    out = np.concatenate([res.results[c]["out"] for c in range(N_CORES)], axis=0)
    return (out + np.asarray(inputs["fc_b"], np.float32)).astype(np.float32)
